# revision 92
# baseline (speedup 1.0000x reference)
"""Cross-modal attention Trainium2 kernel.

Sharding: 8 cores, one per (direction, batch, query-half):
  core = dir*4 + b*2 + qh
  dir 0: out1 rows (q from x1, k/v from x2); dir 1: out2 (q from x2, k/v from x1)
Each core computes a disjoint [1024, 512] slab of one output — no cross-core
reduction. Activations are kept transposed on device ([feature, token]):
  qT/kT = W^T.T @ xT (per 128-feature chunk, heads pairwise stacked 64+64)
  scoresT[j,i] = k_j . q_i  (keys on partitions)
  exp on ScalarE straight from PSUM at FD=1024
The attention loop runs one HEAD per pass (8 passes x 16 key-chunks): one
[128,1024] score tile per step ping-pongs between two PSUM slots so the
ScalarE exp stream never waits on a projection evacuation. attn@v runs with
the exp tile as the matmul STATIONARY ([128 keys, 128 queries] chunks) and v
(+ appended ones-column) as the moving operand, so the accumulator is
[128 queries, 64+denom] — all 128 PSUM partitions carry work (half the PE
cost of the v-stationary orientation) and the softmax denominator lands as a
per-partition scalar: normalize = reciprocal + tensor_scalar_mul, no
partition-broadcast needed. Head pairs 0-2 normalize into paired [128 q, 128] tiles that one XBAR DMA
transpose each flips straight into yat's [feature, token] layout (off the
PE/DVE entirely); the last pair keeps a PE identity-matmul transpose so the
tail is not exposed to the ~2us DMA-path latency.
PSUM: score tiles 2x[128,1024] (4 banks) + 8 attn accumulators packed 7/1
into two [128,512] banks + a 2-slot proj/transpose tag (2 banks) = 8 banks.
Scheduling: v projection (just-in-time in pass 0), the upcoming pairs' q/k
projections (3 per pass), and the previous pass's transposes are spread
through the ACT-bound attention windows; the attn@v flushes trail the exp
stream through a budget-metered FIFO so overloaded early passes smear their
excess into later windows. Seventeen exp tiles in the DVE-light passes run on
the vector engine instead of ScalarE via a Schraudolph bf16 exp
(round(x*2^7/ln2 + 16248) as int16 IS the bf16 bit pattern of ~exp(x); the
denominator sums the same approximation so its ripple largely cancels —
measured +0.4e-3 rel err); their scores route through the projection psum
slots so the score-tile ping-pong never waits on the slower DVE read.
Inputs are shipped p-major so each tensor is ONE dma_start (the ~600ns/start
sequencer cost dominated the old preamble), ordered over the SP/ACT HWDGE +
SWDGE queues so the first score matmul is gated only on {wq,xq,wk,xkv[0:512]}
landing. The output projection's first two feature chunks are pre-accumulated
into SBUF during passes 5-6 and folded back at the tail via identity-matmul
accumulates; the tail interleaves the last head's normalize/transpose with
the remaining projection by query-half, bias-evacuating on the then-idle
ScalarE, and ships the output as bf16.
Biases: q/k folded into the PSUM->SBUF evacuation (per-partition adds);
v bias folded into the output-projection bias on the host (attn rows sum to 1);
1/sqrt(d) folded into Wq/bq on the host.
"""

import sys

sys.path.insert(0, "/opt/trn_rl_repo")

import numpy as np
import ml_dtypes

EMBED = 512
H = 8
D = 64
B = 2
L = 2048
LQ = 1024  # queries per core

_CACHE = {}


def _build_nc(reps=1):
    import concourse.bacc as bacc
    import concourse.mybir as mybir
    import concourse.tile as tile

    BF = mybir.dt.bfloat16
    F32 = mybir.dt.float32
    EXP = mybir.ActivationFunctionType.Exp

    nc = bacc.Bacc("TRN2", target_bir_lowering=False)

    # DRAM I/O, p-major ([128, chunk, n]) so each tensor is one DMA
    xtq = nc.dram_tensor("xtq", [128, 4, LQ], BF, kind="ExternalInput")
    # token-quartered so the first k-projection waits on 1/4 of the transfer
    xtkv = nc.dram_tensor("xtkv", [4, 128, 4, 512], BF, kind="ExternalInput")
    wqt = nc.dram_tensor("wqt", [128, 4, 512], BF, kind="ExternalInput")
    wkt = nc.dram_tensor("wkt", [128, 4, 512], BF, kind="ExternalInput")
    wvt = nc.dram_tensor("wvt", [128, 4, 512], BF, kind="ExternalInput")
    wot = nc.dram_tensor("wot", [128, 4, 512], BF, kind="ExternalInput")
    bqd = nc.dram_tensor("bq", [128, 4], F32, kind="ExternalInput")
    bkd = nc.dram_tensor("bk", [128, 4], F32, kind="ExternalInput")
    bod = nc.dram_tensor("bo", [128, 4], F32, kind="ExternalInput")
    identd = nc.dram_tensor("ident", [128, 128], BF, kind="ExternalInput")
    yt = nc.dram_tensor("yt", [4, 128, LQ], BF, kind="ExternalOutput")

    with tile.TileContext(nc) as tc:
        with tc.tile_pool(name="persist", bufs=1) as persist:
            xq_t = persist.tile([128, 4, LQ], BF, name="xq")
            xkv_q = [
                persist.tile([128, 4, 512], BF, name=f"xkv{t}") for t in range(4)
            ]
            wq_t = persist.tile([128, 4, 512], BF, name="wq")
            wk_t = persist.tile([128, 4, 512], BF, name="wk")
            wv_t = persist.tile([128, 4, 512], BF, name="wv")
            wo_t = persist.tile([128, 4, 512], BF, name="wo")
            bq_t = persist.tile([128, 4], F32, name="bq")
            bk_t = persist.tile([128, 4], F32, name="bk")
            bo_t = persist.tile([128, 4], F32, name="bo")
            id_sb = persist.tile([128, 128], BF, name="id")
            xq_sb = [xq_t[:, c, :] for c in range(4)]
            wq_sb = [wq_t[:, c, :] for c in range(4)]
            wk_sb = [wk_t[:, c, :] for c in range(4)]
            wv_sb = [wv_t[:, c, :] for c in range(4)]
            wo_sb = [wo_t[:, c, :] for c in range(4)]
            bq_sb = [bq_t[:, c : c + 1] for c in range(4)]
            bk_sb = [bk_t[:, c : c + 1] for c in range(4)]
            bo_sb = [bo_t[:, c : c + 1] for c in range(4)]
            qt_sb = [persist.tile([128, LQ], BF, name=f"qt{f}") for f in range(4)]
            kt_sb = [persist.tile([128, L], BF, name=f"kt{f}") for f in range(4)]
            # v in natural layout, per 128-token chunk, heads strided by 65 so
            # each head slice [128, 65] carries its ones-column (softmax denom)
            v_sb = [persist.tile([128, H, D + 1], BF, name=f"v{l}") for l in range(16)]
            yat_sb = [persist.tile([128, LQ], BF, name=f"yat{f}") for f in range(4)]
            # output-projection partials over feature chunks 0-1 (heads 0-3),
            # spilled to SBUF during passes 5-6 and folded back at the tail
            yp_sb = [persist.tile([128, 512], BF, name=f"yp{k}") for k in range(8)]

            for _rep in range(reps):
                # The sim's DMA device is serial FIFO by descriptor-gen
                # completion, and a consumer waits on the per-queue DMA
                # watermark at its emission point. Critical prefix: bulk
                # {wq,xq,wk} on HWDGE (serial gen), everything else via
                # SWDGE whose descriptor-gen runs in parallel on Pool;
                # remaining inputs are issued mid-loop so early consumers'
                # watermarks stay small.
                nc.sync.dma_start(out=wq_t, in_=wqt[:, :, :])
                nc.scalar.dma_start(out=xq_t, in_=xtq[:, :, :])
                nc.sync.dma_start(out=wk_t, in_=wkt[:, :, :])
                nc.gpsimd.dma_start(out=bq_t, in_=bqd[:, :])
                nc.gpsimd.dma_start(out=bk_t, in_=bkd[:, :])
                # memsets here delay the xkv descriptor-gen just enough that
                # the quarters don't jump ahead of xq/wk in the DMA FIFO
                for l in range(16):
                    nc.gpsimd.memset(v_sb[l][:, :, D : D + 1], 1.0)
                nc.gpsimd.dma_start(out=xkv_q[0], in_=xtkv[0])
                nc.gpsimd.dma_start(out=xkv_q[1], in_=xtkv[1])
                nc.gpsimd.dma_start(out=wv_t, in_=wvt[:, :, :])
                late_dma = {
                    0: [lambda: nc.gpsimd.dma_start(out=xkv_q[2], in_=xtkv[2])],
                    2: [lambda: nc.gpsimd.dma_start(out=xkv_q[3], in_=xtkv[3])],
                    5: [lambda: nc.gpsimd.dma_start(out=wo_t, in_=wot[:, :, :])],
                    6: [
                        lambda: nc.gpsimd.dma_start(out=bo_t, in_=bod[:, :]),
                        lambda: nc.gpsimd.dma_start(out=id_sb, in_=identd[:, :]),
                    ],
                }

                with (
                    tc.tile_pool(name="scps", bufs=2, space="PSUM") as scps,
                    tc.tile_pool(name="avps", bufs=1, space="PSUM") as avps,
                    tc.tile_pool(name="pps", bufs=2, space="PSUM") as pps,
                    tc.tile_pool(name="att", bufs=3) as att,
                    tc.tile_pool(name="nrm", bufs=2) as nrm,
                ):
                    # 8 accumulators of [128, 65] packed 7 + 1 into two banks
                    ava = avps.tile([128, 512], F32, name="ava")
                    avb = avps.tile([128, 512], F32, name="avb")

                    def av_slot(qc):
                        return (ava, qc * 65) if qc < 7 else (avb, 0)

                    # prime the ScalarE exp table load during the DMA phase
                    dm = nrm.tile([1, 2], F32, name="dm")
                    nc.vector.memset(dm, 0.0)
                    dm2 = nrm.tile([1, 2], F32, name="dm2")
                    nc.scalar.activation(dm2, dm, EXP)
                    # warm the PE clock (the cost model un-throttles after
                    # ~3us of sustained matmul activity) while inputs land
                    wup = nrm.tile([128, 512], BF, name="wup")
                    nc.vector.memset(wup, 0.0)
                    wps = scps.tile([128, 512], F32, name="sc")
                    for i in range(7):
                        nc.tensor.matmul(
                            wps, wup[:, 0:128], wup, start=(i == 0), stop=(i == 6)
                        )

                    qk_ps = [None]

                    def qk_half(f, g, half):
                        # g 0..1: q i-halves; g 2..5: k quarters; each group
                        # is two 2-matmul halves so a filler step never
                        # overflows the 1038ns exp window
                        if half == 0:
                            qk_ps[0] = pps.tile([128, 512], F32, name="ps")
                        ps = qk_ps[0]
                        ih = g if g < 2 else g - 2
                        for c in (0, 1) if half == 0 else (2, 3):
                            if g < 2:
                                nc.tensor.matmul(
                                    ps,
                                    wq_sb[c][:, f * 128 : (f + 1) * 128],
                                    xq_sb[c][:, ih * 512 : (ih + 1) * 512],
                                    start=(c == 0),
                                    stop=(c == 3),
                                )
                            else:
                                nc.tensor.matmul(
                                    ps,
                                    wk_sb[c][:, f * 128 : (f + 1) * 128],
                                    xkv_q[ih][:, c, :],
                                    start=(c == 0),
                                    stop=(c == 3),
                                )
                        if half == 1:
                            dst, bias = (
                                (qt_sb, bq_sb) if g < 2 else (kt_sb, bk_sb)
                            )
                            nc.vector.tensor_scalar_add(
                                dst[f][:, ih * 512 : (ih + 1) * 512], ps, bias[f]
                            )

                    def qk_group(f, g):
                        qk_half(f, g, 0)
                        qk_half(f, g, 1)

                    def v_proj(l):
                        ps = pps.tile([128, 512], F32, name="ps")
                        for c in range(4):
                            nc.tensor.matmul(
                                ps,
                                xkv_q[l // 4][:, c, (l % 4) * 128 : (l % 4 + 1) * 128],
                                wv_sb[c],
                                start=(c == 0),
                                stop=(c == 3),
                            )
                        if l <= 9:  # ScalarE has bubbles in the PE-bound
                            # early passes; DVE carries the deferred rest
                            nc.scalar.copy(
                                v_sb[l][:, :, 0:D],
                                ps.rearrange("p (h d) -> p h d", h=H),
                            )
                        else:
                            nc.vector.tensor_copy(
                                v_sb[l][:, :, 0:D],
                                ps.rearrange("p (h d) -> p h d", h=H),
                            )

                    tp_q = []  # pending (head, qc, avn) transposes

                    def emit_tp(n):
                        for _ in range(n):
                            if not tp_q:
                                return
                            kind, ph, qc, avn = tp_q.pop(0)
                            if kind == "dma":
                                nc.sync.dma_start_transpose(
                                    out=yat_sb[ph][
                                        :, qc * 128 : (qc + 1) * 128
                                    ],
                                    in_=avn,
                                )
                                continue
                            tpt = pps.tile([128, 128], BF, name="ps")
                            nc.tensor.transpose(tpt[0:64, :], avn, id_sb)
                            nc.vector.tensor_copy(
                                yat_sb[ph // 2][
                                    (ph % 2) * 64 : (ph % 2) * 64 + 64,
                                    qc * 128 : (qc + 1) * 128,
                                ],
                                tpt[0:64, :],
                            )

                    av2_hold = {}

                    def norm(h, qc, on_act=False):
                        avt, o = av_slot(qc)
                        rcol = nrm.tile([128, 1], F32, name="rc", bufs=4)
                        nc.vector.reciprocal_approx_fast(
                            out=rcol, in_=avt[:, o + 64 : o + 65]
                        )
                        if h < 6:
                            # head pairs 0-2: normalize into a paired
                            # [128, 128] tile (even head cols 0:64, odd
                            # 64:128); one XBAR DMA then transposes the
                            # whole [q, d-pair] block straight into yat,
                            # replacing a PE transpose + DVE evac each
                            if h % 2 == 0:
                                av2 = nrm.tile([128, 128], BF, name="av2",
                                               bufs=12)
                                av2_hold[qc] = av2
                            av2 = av2_hold[qc]
                            dst = av2[:, (h % 2) * 64 : (h % 2) * 64 + 64]
                            nc.vector.tensor_scalar_mul(
                                dst, avt[:, o : o + 64], rcol
                            )
                            if h % 2 == 1:
                                tp_q.append(("dma", h // 2, qc, av2))
                            return
                        avn = nrm.tile([128, D], BF, name="avn", bufs=10)
                        if on_act:  # ScalarE is idle at the tail
                            nc.scalar.mul(avn, avt[:, o : o + 64], rcol)
                        else:
                            nc.vector.tensor_scalar_mul(
                                avn, avt[:, o : o + 64], rcol
                            )
                        tp_q.append(("pe", h, qc, avn))

                    # qk projection half-groups for upcoming pairs, spread
                    # over passes (pass 0 also finishes its k quarters 1..3)
                    fillers = {}

                    def put_qk(t, f, g):
                        fillers.setdefault(t, []).append(("qk", f, g, 0))
                        fillers.setdefault(t + 1, []).append(("qk", f, g, 1))

                    put_qk(1, 0, 3)
                    put_qk(5, 0, 4)
                    put_qk(9, 0, 5)
                    for g in range(6):
                        put_qk(16 + 2 * g + 1, 1, g)
                        put_qk(32 + 16 * (g // 3) + 4 * (g % 3) + 1, 2, g)
                        put_qk(64 + 16 * (g // 3) + 4 * (g % 3) + 1, 3, g)
                    for k in range(8):  # out-proj ci=0,1 partials
                        fillers.setdefault(
                            80 + 16 * (k // 4) + 4 * (k % 4) + 2, []
                        ).append(("op", k // 2, k % 2))

                    def op_partial(co, ih):
                        ps = pps.tile([128, 512], F32, name="ps")
                        for ci in range(2):
                            nc.tensor.matmul(
                                ps,
                                wo_sb[ci][:, co * 128 : (co + 1) * 128],
                                yat_sb[ci][:, ih * 512 : (ih + 1) * 512],
                                start=(ci == 0),
                                stop=(ci == 1),
                            )
                        nc.vector.tensor_copy(yp_sb[co * 2 + ih], ps)

                    def av_flush(pex, ph, pj):
                        # a start=True matmul wipes its whole PSUM bank, so
                        # only the FIRST group per bank starts (qc 0 on ava,
                        # qc 7 on avb); the rest land on has_written=0
                        # (overwrite) at j=0 and accumulate afterwards
                        for qc in range(8):
                            avt, o = av_slot(qc)
                            nc.tensor.matmul(
                                avt[:, o : o + 65],
                                pex[:, qc * 128 : (qc + 1) * 128],
                                v_sb[pj][:, ph, :],
                                start=(pj == 0 and qc in (0, 7)),
                                stop=(pj == 15),
                                skip_group_check=True,
                            )

                    # the attn@v flushes (and pass-0's just-in-time v
                    # projections) trail the exp stream through a FIFO,
                    # metered by a per-step PE-time budget so the overloaded
                    # early passes smear their excess into later ACT-bound
                    # windows instead of stalling the exp stream
                    ex_q = []
                    v_next = [0]
                    # start in debt: the preamble's projections have first
                    # claim on PE, and v0 must not be hoisted ahead of the
                    # first scores (it waits on the late wv DMA)
                    bal = [-2500.0]

                    def pop_flush():
                        fex, fh, fj = ex_q.pop(0)
                        if fh == 0:
                            while v_next[0] <= min(fj + 1, 15):
                                v_proj(v_next[0])
                                v_next[0] += 1
                                bal[0] -= 852
                        av_flush(fex, fh, fj)
                        bal[0] -= 216
                        if fj == 15 and fh < 7:
                            for qc in range(8):
                                norm(fh, qc)

                    # preamble: q halves, a ramp-keeper, then the first k
                    # quarter split so scores j=0 only wait on 128 tokens
                    qk_group(0, 0)
                    qk_group(0, 1)
                    wu2 = scps.tile([128, 512], F32, name="sc")
                    for i in range(5):
                        nc.tensor.matmul(
                            wu2, wup[:, 0:128], wup, start=(i == 0), stop=(i == 4)
                        )
                    for lo, hi in ((0, 128), (128, 256), (256, 512)):
                        ps0 = pps.tile([128, 512], F32, name="ps")
                        for c in range(4):
                            nc.tensor.matmul(
                                ps0[:, 0 : hi - lo],
                                wk_sb[c][:, 0:128],
                                xkv_q[0][:, c, lo:hi],
                                start=(c == 0),
                                stop=(c == 3),
                            )
                        nc.vector.tensor_scalar_add(
                            kt_sb[0][:, lo:hi], ps0[:, 0 : hi - lo], bk_sb[0]
                        )

                    for h in range(8):  # one head per pass
                        fc, hp = h // 2, (h % 2) * 64
                        for j in range(16):  # key chunks
                            ex = att.tile([128, LQ], BF, name="ex", bufs=21)
                            if (h, j) in ((5, 14), (6, 14), (3, 13), (4, 12), (4, 14), (5, 12), (6, 5), (6, 9), (6, 12), (7, 5), (7, 7), (7, 9), (7, 12), (7, 14)):
                                # DVE-light passes: take whole exp tiles off
                                # the ScalarE stream. Scores go through the
                                # pps slots (quiet here) so the sc ping-pong
                                # never stalls on the slower DVE read, and
                                # exp is the Schraudolph bf16 approximation:
                                # round(x*2^7/ln2 + B) as int16 IS the bf16
                                # bit pattern of ~exp(x); the denominator
                                # sums the same approximation so the scale
                                # ripple largely cancels.
                                for ih in range(2):
                                    psh = pps.tile([128, 512], F32, name="ps")
                                    nc.tensor.matmul(
                                        psh,
                                        kt_sb[fc][
                                            hp : hp + 64, j * 128 : (j + 1) * 128
                                        ],
                                        qt_sb[fc][
                                            hp : hp + 64, ih * 512 : (ih + 1) * 512
                                        ],
                                        start=True,
                                        stop=True,
                                    )
                                    nc.vector.tensor_scalar(
                                        out=ex.bitcast(mybir.dt.int16)[
                                            :, ih * 512 : (ih + 1) * 512
                                        ],
                                        in0=psh,
                                        scalar1=184.6650,
                                        scalar2=16248.0,
                                        op0=mybir.AluOpType.mult,
                                        op1=mybir.AluOpType.add,
                                    )
                            else:
                                sc = scps.tile([128, LQ], F32, name="sc")
                                for ih in range(2):
                                    nc.tensor.matmul(
                                        sc[:, ih * 512 : (ih + 1) * 512],
                                        kt_sb[fc][
                                            hp : hp + 64, j * 128 : (j + 1) * 128
                                        ],
                                        qt_sb[fc][
                                            hp : hp + 64, ih * 512 : (ih + 1) * 512
                                        ],
                                        start=True,
                                        stop=True,
                                    )
                                nc.scalar.activation(ex, sc, EXP)
                            bal[0] += 1038 - 426
                            if h == 0:
                                for thunk in late_dma.get(j, ()):
                                    thunk()
                            had_qk = False
                            for task in fillers.get(h * 16 + j, ()):
                                if task[0] == "qk":
                                    qk_half(task[1], task[2], task[3])
                                else:
                                    op_partial(task[1], task[2])
                                bal[0] -= 426
                                had_qk = True
                            emit_tp(1)
                            ex_q.append((ex, h, j))
                            cap = (20, 18, 15, 13, 11, 9, 4, 2)[h]
                            while ex_q and (
                                (bal[0] > 700 and not had_qk) or len(ex_q) > cap
                            ):
                                pop_flush()
                            if not ex_q:
                                bal[0] = min(bal[0], 1000.0)
                    while ex_q:
                        pop_flush()

                    # ---- tail: last head's normalize/transpose interleaved
                    # with the output projection, by query-half --------------
                    with tc.tile_pool(name="yst", bufs=1) as yst:
                        yts_l = [
                            yst.tile([128, LQ], BF, name=f"yts{co}")
                            for co in range(4)
                        ]
                        for ih in range(2):  # query halves
                            w = slice(ih * 512, (ih + 1) * 512)
                            for qc in range(4 * ih, 4 * ih + 4):
                                norm(7, qc, on_act=(ih == 0))
                            emit_tp(len(tp_q))
                            for co in range(4):
                                ps = pps.tile([128, 512], F32, name="ps")
                                for ci in range(2, 4):
                                    nc.tensor.matmul(
                                        ps,
                                        wo_sb[ci][:, co * 128 : (co + 1) * 128],
                                        yat_sb[ci][:, w],
                                        start=(ci == 2),
                                        stop=False,
                                    )
                                # fold the spilled ci=0,1 partial back in via
                                # an identity-matmul accumulate
                                nc.tensor.matmul(
                                    ps, id_sb, yp_sb[co * 2 + ih],
                                    start=False, stop=True,
                                )
                                # bias evac on ScalarE: it is idle after the
                                # last exp, while DVE still has norm work
                                nc.scalar.activation(
                                    yts_l[co][:, w],
                                    ps,
                                    mybir.ActivationFunctionType.Identity,
                                    bias=bo_sb[co],
                                )
                                # the ~0.6us/dma_start SEQ cost splits over
                                # two sequencers (SP + SWDGE)
                                nc.sync.dma_start(
                                    out=yt[co][:, w], in_=yts_l[co][:, w]
                                )

    nc.finalize()
    return nc


def _pmajor(a, nchunk):
    # [nchunk*128, n] -> [128, nchunk, n]
    return np.ascontiguousarray(a.reshape(nchunk, 128, -1).transpose(1, 0, 2))


def _prep_weights(qkv_w, qkv_b, out_w, out_b):
    bf = ml_dtypes.bfloat16
    w = qkv_w.reshape(H, 3, D, EMBED)
    b3 = qkv_b.reshape(H, 3, D)
    scale = 1.0 / np.sqrt(D).astype(np.float32)
    wq = w[:, 0].reshape(EMBED, EMBED) * scale
    wk = w[:, 1].reshape(EMBED, EMBED)
    wv = w[:, 2].reshape(EMBED, EMBED)
    bq = (b3[:, 0].reshape(EMBED) * scale).astype(np.float32)
    bk = b3[:, 1].reshape(EMBED).astype(np.float32)
    bv = b3[:, 2].reshape(EMBED).astype(np.float32)
    out = {
        "wqt": _pmajor(np.ascontiguousarray(wq.T), 4).astype(bf),
        "wkt": _pmajor(np.ascontiguousarray(wk.T), 4).astype(bf),
        "wvt": _pmajor(np.ascontiguousarray(wv.T), 4).astype(bf),
        "wot": _pmajor(np.ascontiguousarray(out_w.T), 4).astype(bf),
        "bq": np.ascontiguousarray(bq.reshape(4, 128).T),
        "bk": np.ascontiguousarray(bk.reshape(4, 128).T),
        "bo": np.ascontiguousarray(
            (out_b + out_w @ bv).astype(np.float32).reshape(4, 128).T
        ),
        "ident": np.eye(128, dtype=np.float32).astype(bf),
    }
    return out


def _make_in_maps(x1, x2, qkv_w, qkv_b, out_w, out_b):
    x1 = np.asarray(x1, dtype=np.float32)
    x2 = np.asarray(x2, dtype=np.float32)
    shared = _prep_weights(
        np.asarray(qkv_w, np.float32),
        np.asarray(qkv_b, np.float32),
        np.asarray(out_w, np.float32),
        np.asarray(out_b, np.float32),
    )

    bf = ml_dtypes.bfloat16
    xT = {
        0: [np.ascontiguousarray(x1[b].T).astype(bf) for b in range(B)],  # [512, L]
        1: [np.ascontiguousarray(x2[b].T).astype(bf) for b in range(B)],
    }

    in_maps = []
    for core in range(8):
        d, b, qh = core // 4, (core // 2) % 2, core % 2
        xq_mod = d  # dir 0 -> q from x1
        xkv_mod = 1 - d
        m = dict(shared)
        m["xtq"] = _pmajor(
            np.ascontiguousarray(xT[xq_mod][b][:, qh * LQ : (qh + 1) * LQ]), 4
        )
        # [512, 2048] -> [tq, 128, c, 512]
        m["xtkv"] = np.ascontiguousarray(
            xT[xkv_mod][b].reshape(4, 128, 4, 512).transpose(2, 1, 0, 3)
        )
        in_maps.append(m)
    return in_maps


def kernel(x1, x2, qkv_w, qkv_b, out_w, out_b):
    from concourse.bass_utils import run_bass_kernel_spmd

    in_maps = _make_in_maps(x1, x2, qkv_w, qkv_b, out_w, out_b)

    if "nc" not in _CACHE:
        _CACHE["nc"] = _build_nc()
    try:
        res = run_bass_kernel_spmd(_CACHE["nc"], in_maps, core_ids=list(range(8)))
    except Exception:
        # transient runtime hiccups (e.g. a stale device state) recover on retry
        res = run_bass_kernel_spmd(_CACHE["nc"], in_maps, core_ids=list(range(8)))

    out1 = np.empty((B, L, EMBED), np.float32)
    out2 = np.empty((B, L, EMBED), np.float32)
    outs = {0: out1, 1: out2}
    for core in range(8):
        d, b, qh = core // 4, (core // 2) % 2, core % 2
        ytc = res.results[core]["yt"].astype(np.float32).reshape(512, LQ)
        outs[d][b, qh * LQ : (qh + 1) * LQ, :] = ytc.T
    return out1, out2
